# revision 53
# baseline (speedup 1.0000x reference)
"""ContextualAttention2D Trainium2 kernel.

Full inputs -> full output; internally data-parallel over batch across 8
NeuronCores (2 batches per core), single SPMD NEFF, no collectives.

Math (per batch):
  hidden[n,c]   = x.reshape(C, H*W).T
  hn            = layernorm_c(hidden) * ln_w + ln_b
  q             = hn @ Wq.T ;  k = ctx @ Wk.T ; v = ctx @ Wv.T
  ctx           = context @ Wctx.T      (folded: k = context @ (Wk@Wctx).T etc)
  attn          = softmax_l(q @ k.T * hd^-0.5 + maskbias) ; out = attn @ v
  y             = (out @ Wo.T + hidden).T.reshape(C, H, W)

fp8 (e4m3) DoubleRow matmuls carry the projection GEMMs and attn@V at
0.5 cycles/row with two 128-deep k-tiles per instruction.  Per-tensor
power-of-two scales keep every fp8 operand in range:
  wq8 = 64*Wq*ln_w/8          q_psum = 64*q    rbc = rstd/1024 (Rsqrt scale)
  wck8 = 16*(Wk@Wctx).T       k_sb  = 16*k     (1/16 cancelled by rbc)
  wcv8 = 16*(Wv@Wctx).T       v8    = 16*v     aug ones col = 1/8
  probs = exp(scores+mb) e4m3 den8 = den/8     rcb2 = 8/den  -> an8 = 128*attn
  wo8 = 32*Wo.T               out_psum = 4096*out, residual add scales 1/4096

LayerNorm: per-token mean/var via ones-matmuls (cross-partition sum);
rstd/1024 from ACT Rsqrt (scale=2^20), invr via DVE (var+eps)*rstd; the
mean correction enters Q as a rank-2 bf16 matmul into the same PSUM group.

Softmax denominator: attn@V is augmented with a 1/8 ones column; the
denominator row is DMA-gathered into a [128,32] tile so one DVE
reciprocal covers all 8 heads, then DMA-broadcast back per head-pair.
attn@V PSUM is evicted by DMA (off-engine) and normalized straight into
the fp8 out-projection operand.
"""
import numpy as np
import ml_dtypes

from concourse import bacc, mybir, tile
from concourse.bass_utils import run_bass_kernel_spmd

BF = ml_dtypes.bfloat16
F8 = ml_dtypes.float8_e4m3

B, C, H, W = 16, 512, 32, 32
NH, HD = 8, 64
CTX_DIM, L = 768, 512
EPS = 1e-5
N = H * W                 # 1024 tokens
NCORES = 8
BPC = B // NCORES         # batches per core
P = 128
CC = C // P               # 4 c-chunks
DC = CTX_DIM // P         # 6 d-chunks
LC = L // P               # 4 l-chunks
MC = N // 512             # 2 token chunks of 512
MASK_NEG = -30000.0

F32 = mybir.dt.float32
BF16 = mybir.dt.bfloat16
FP8 = mybir.dt.float8e4
DR = mybir.MatmulPerfMode.DoubleRow

_NC_CACHE = None


def _build():
    nc = bacc.Bacc(None, target_bir_lowering=False, debug=False)

    x8d = nc.dram_tensor("x8", [BPC, C, N], FP8, kind="ExternalInput")
    xbfd = nc.dram_tensor("xbf", [BPC, C, N], BF16, kind="ExternalInput")
    ctx8d = nc.dram_tensor("ctx8", [BPC, CTX_DIM, L], FP8, kind="ExternalInput")
    # fp8 aug rows: mask row for k8 (i=0: -224 masked / 0, i=1: zeros) and
    # the constant 16 / 0 rows for q8
    mrowd = nc.dram_tensor("mrow8", [BPC, 1, NH, L], FP8, kind="ExternalInput")
    qaugd = nc.dram_tensor("qaug8", [1, NH, MC, 512], FP8, kind="ExternalInput")
    wq8d = nc.dram_tensor("wq8", [C, C], FP8, kind="ExternalInput")
    wck8d = nc.dram_tensor("wck8", [CTX_DIM, C], FP8, kind="ExternalInput")
    wcv8d = nc.dram_tensor("wcv8", [CTX_DIM, C], FP8, kind="ExternalInput")
    wo8d = nc.dram_tensor("wo8", [C, C], FP8, kind="ExternalInput")
    qr2d = nc.dram_tensor("q_r2", [2, C], BF16, kind="ExternalInput")
    yd = nc.dram_tensor("y", [BPC, C, N], F32, kind="ExternalOutput")

    with tile.TileContext(nc) as tc:
        with (
            tc.tile_pool(name="wpool", bufs=1) as wpool,
            tc.tile_pool(name="xpool", bufs=2) as xpool,
            tc.tile_pool(name="actpool", bufs=2) as actpool,
            tc.tile_pool(name="ppool", bufs=6) as ppool,
            tc.tile_pool(name="spool", bufs=2) as spool,
            tc.tile_pool(name="psum", bufs=2, space="PSUM") as psum,
            tc.tile_pool(name="psc", bufs=2, space="PSUM") as psc,
            tc.tile_pool(name="paug", bufs=2, space="PSUM") as paug,
            tc.tile_pool(name="dpool", bufs=4, space="DRAM") as dpool,
        ):
            # ---- persistent weights ----
            wq_sb = wpool.tile([P, CC, C], FP8)
            nc.scalar.dma_start(wq_sb[:], wq8d.ap().rearrange("(cc p) e -> p cc e", p=P))
            wck_sb = wpool.tile([P, DC, C], FP8)
            nc.scalar.dma_start(wck_sb[:], wck8d.ap().rearrange("(dc p) e -> p dc e", p=P))
            wcv_sb = wpool.tile([P, DC, C], FP8)
            nc.scalar.dma_start(wcv_sb[:], wcv8d.ap().rearrange("(dc p) e -> p dc e", p=P))
            wo_sb = wpool.tile([P, CC, C], FP8)
            nc.scalar.dma_start(wo_sb[:], wo8d.ap().rearrange("(ec p) c -> p ec c", p=P))
            qr2_sb = wpool.tile([2, C], BF16)
            nc.scalar.dma_start(qr2_sb[:], qr2d.ap())

            ones1_sb = wpool.tile([P, 1], BF16)   # stats lhsT (column sums)
            nc.vector.memset(ones1_sb[:], 1.0)
            onesr_sb = wpool.tile([1, P], BF16)    # bcast-matmul lhsT (rank-1)
            nc.vector.memset(onesr_sb[:], 1.0)
            eps2_sb = wpool.tile([1, 1], F32)      # eps * 2^8 (scaled Sqrt bias)
            nc.vector.memset(eps2_sb[:], EPS * 256.0)

            # Per-batch emission closures; emitted in a software-pipelined
            # order so PE filler (projection chains) sits between the
            # ACT-bound score-exp groups and their attn@v consumers.
            def make_batch(b):
                st = {}

                def loads():
                    # spread bulk loads across queues so ctx (feeds v/k), xbf
                    # (feeds stats) and x8 (feeds q) all stream in parallel;
                    # b0's ctx rides sync, later batches keep off sync so the
                    # previous batch's latency-sensitive normalize DMAs win.
                    bulk = nc.sync.dma_start if b == 0 else nc.gpsimd.dma_start
                    st["x8"] = xpool.tile([P, CC, N], FP8, name=f"x8{b}", tag="x8")
                    st["xbf"] = xpool.tile([P, CC, N], BF16, name=f"xbf{b}", tag="xbf")
                    st["ctx8"] = xpool.tile([P, DC, L], FP8, name=f"ctx8{b}", tag="ctx8")
                    for dc in range(DC):
                        bulk(st["ctx8"][:, dc, :],
                             ctx8d.ap()[b][dc * P:(dc + 1) * P, :])
                    for cc in range(CC):
                        nc.gpsimd.dma_start(st["xbf"][:, cc, :],
                                            xbfd.ap()[b][cc * P:(cc + 1) * P, :])
                    for cc in range(CC):
                        nc.scalar.dma_start(
                            st["x8"][:, cc, :],
                            x8d.ap()[b][cc * P:(cc + 1) * P, :])
                    # scores operands: [65, h, ...] fp8, aug row 64 carries the
                    # mask on the k side and a constant 16 on the q side.
                    # Head dims are host-interleaved so one [128,512] DMA
                    # fills both heads of an ec chunk (row r -> head r%2).
                    st["k8"] = actpool.tile([65, NH, L], FP8,
                                            name=f"k8{b}", tag="k8")
                    st["q8"] = actpool.tile([65, NH, MC, 512], FP8,
                                            name=f"q8{b}", tag="q8")
                    nc.sync.dma_start(st["k8"][64:65, :, :], mrowd.ap()[b])
                    nc.sync.dma_start(st["q8"][64:65, :, :, :], qaugd.ap()[0])
                    st["xsq"] = xpool.tile([P, CC, N], BF16, name=f"xsq{b}",
                                           tag="xsq", bufs=1)
                    for cc in range(CC):
                        nc.gpsimd.tensor_tensor(
                            st["xsq"][:, cc, :], st["xbf"][:, cc, :],
                            st["xbf"][:, cc, :], op=mybir.AluOpType.mult)

                    # v8: [d, lc-pair u, k-tile i, head, 96]; col 64 = 1/8 ones
                    # (denominator), cols 65:96 zero pad (DoubleRow stationary
                    # width must be a multiple of 32)
                    st["v8"] = actpool.tile([P, LC // 2, 2, NH, 96], FP8,
                                            name=f"v8{b}", tag="v8")
                    nc.vector.memset(st["v8"][:, :, :, :, HD + 1:], 0.0)
                    nc.vector.memset(st["v8"][:, :, :, :, HD:HD + 1], 0.125)
                    st["an8"] = actpool.tile([P, CC, MC, 512], FP8,
                                             name=f"an8{b}", tag="an8")
                    st["r2"] = {}
                    st["rbc"] = {}
                    st["den"] = {}
                    st["asb"] = {}
                    st["rcb"] = {}

                def stats(mc):
                    ms = slice(mc * 512, (mc + 1) * 512)
                    st1 = psum.tile([1, 512], F32, name=f"st1{b}{mc}", tag="ps")
                    for cc in range(CC):
                        nc.tensor.matmul(st1[:], ones1_sb[:], st["xbf"][:, cc, ms],
                                         start=(cc == 0), stop=(cc == CC - 1))
                    st2 = psum.tile([1, 512], F32, name=f"st2{b}{mc}", tag="ps")
                    for cc in range(CC):
                        nc.tensor.matmul(st2[:], ones1_sb[:], st["xsq"][:, cc, ms],
                                         start=(cc == 0), stop=(cc == CC - 1))
                    negmu = spool.tile([1, 512], BF16, name=f"negmu{b}{mc}", tag="negmu")
                    nc.vector.tensor_scalar_mul(negmu[:], st1[:], -1.0 / C)
                    musq = spool.tile([1, 512], F32, name=f"musq{b}{mc}", tag="musq")
                    nc.vector.tensor_tensor(musq[:], negmu[:], negmu[:],
                                            op=mybir.AluOpType.mult)
                    var = spool.tile([1, 512], F32, name=f"var{b}{mc}", tag="var")
                    nc.vector.scalar_tensor_tensor(
                        var[:], st2[:], 1.0 / C, musq[:],
                        op0=mybir.AluOpType.mult, op1=mybir.AluOpType.subtract)
                    # invr_k = sqrt((var+eps)*2^8) = 16*invr  (fp32 for recip)
                    invr_k = spool.tile([1, 512], F32, name=f"invk{b}{mc}", tag="invk")
                    nc.scalar.activation(invr_k[:], var[:],
                                         mybir.ActivationFunctionType.Sqrt,
                                         bias=eps2_sb[:], scale=256.0)
                    rstd_f = spool.tile([1, 512], F32, name=f"rstf{b}{mc}", tag="rstf")
                    nc.vector.reciprocal_approx_fast(rstd_f[:], invr_k[:])
                    rstd = spool.tile([1, 512], BF16, name=f"rstd{b}{mc}", tag="rstd")
                    nc.vector.tensor_copy(rstd[:], rstd_f[:])
                    # invr/16 = (var+eps) * (rstd/16)
                    invr = spool.tile([1, 512], BF16, name=f"invr{b}{mc}", tag="invr")
                    nc.vector.scalar_tensor_tensor(
                        invr[:], var[:], EPS, rstd[:],
                        op0=mybir.AluOpType.add, op1=mybir.AluOpType.mult)
                    r2 = spool.tile([2, 512], BF16, name=f"r2_{b}{mc}", tag="r2")
                    nc.sync.dma_start(r2[0:1, :], negmu[:])
                    nc.sync.dma_start(r2[1:2, :], invr[:])
                    rbp = paug.tile([P, 512], F32, name=f"rbp{b}{mc}", tag="aug")
                    nc.tensor.matmul(rbp[:], onesr_sb[:], rstd[:],
                                     start=True, stop=True)
                    rbc = spool.tile([P, 512], BF16, name=f"rbc{b}{mc}", tag="rbc")
                    nc.vector.tensor_copy(rbc[:], rbp[:])
                    st["r2"][mc] = r2
                    st["rbc"][mc] = rbc

                def k_chain(ec):
                    es = slice(ec * P, (ec + 1) * P)
                    kp = psum.tile([P, 512], F32, name=f"kp{b}{ec}", tag="ps")
                    for u in range(DC // 2):
                        nc.tensor.matmul(kp[:], wck_sb[:, 2 * u:2 * u + 2, es],
                                         st["ctx8"][:, 2 * u:2 * u + 2, :],
                                         start=(u == 0), stop=(u == DC // 2 - 1),
                                         perf_mode=DR)
                    ktmp = ppool.tile([P, 512], FP8, name=f"ktmp{b}{ec}",
                                      tag="ktmp", bufs=3)
                    with nc.allow_low_precision(reason="fp8 attn operand; error damped by residual"):
                        nc.vector.tensor_copy(ktmp[:], kp[:])
                    # partition reshuffle rides the DMA engines
                    # (k8[p, 2ec + r%2] = ktmp[2p + r%2] via host interleave)
                    nc.gpsimd.dma_start(
                        st["k8"][0:HD, 2 * ec:2 * ec + 2, :], ktmp[:])

                def v_chain(lc):
                    ls = slice(lc * P, (lc + 1) * P)
                    vp = psum.tile([P, 512], F32, name=f"vp{b}{lc}", tag="ps")
                    for u in range(DC // 2):
                        nc.tensor.matmul(vp[:], st["ctx8"][:, 2 * u:2 * u + 2, ls],
                                         wcv_sb[:, 2 * u:2 * u + 2, :],
                                         start=(u == 0), stop=(u == DC // 2 - 1),
                                         perf_mode=DR)
                    with nc.allow_low_precision(reason="fp8 attn values; error damped by residual"):
                        nc.vector.tensor_copy(
                            st["v8"][:, lc // 2, lc % 2, :, 0:HD],
                            vp[:].rearrange("p (h d) -> p h d", d=HD))

                def q_chain(ec, mc):
                    es = slice(ec * P, (ec + 1) * P)
                    ms = slice(mc * 512, (mc + 1) * 512)
                    qp = psum.tile([P, 512], F32, name=f"qp{b}{ec}{mc}", tag="ps")
                    for u in range(CC // 2):
                        nc.tensor.matmul(qp[:], wq_sb[:, 2 * u:2 * u + 2, es],
                                         st["x8"][:, 2 * u:2 * u + 2, ms],
                                         start=(u == 0), stop=False,
                                         perf_mode=DR)
                    nc.tensor.matmul(qp[:], qr2_sb[:, es], st["r2"][mc][:],
                                     start=False, stop=True)
                    qtmp = ppool.tile([P, 512], FP8, name=f"qtmp{b}{ec}{mc}",
                                      tag="qtmp", bufs=3)
                    with nc.allow_low_precision(reason="fp8 attn operand; error damped by residual"):
                        nc.vector.tensor_tensor(qtmp[:], qp[:],
                                                st["rbc"][mc][:],
                                                op=mybir.AluOpType.mult)
                    nc.gpsimd.dma_start(
                        st["q8"][0:HD, 2 * ec:2 * ec + 2, mc, :], qtmp[:])

                def sc_exp_group(mc, j):
                    if mc not in st["den"]:
                        st["den"][mc] = spool.tile([NH, 512], F32,
                                                   name=f"den{b}{mc}", tag="den")
                        st["asb"][mc] = {}
                    # probs for this head-pair: per hh a [P, 2(u: lc pair),
                    # 2(i), 512] fp8 tile feeding the DoubleRow attn@v.
                    # scores are DoubleRow too ([33,2,*] k/q with the mask and
                    # a 16-const in aug row 32); exp applies the 1/64 descale.
                    ps_h = []
                    for hh in range(2):
                        h = 2 * j + hh
                        p8 = ppool.tile([P, 2, 2, 512], FP8,
                                        name=f"p8{b}{mc}{j}{hh}", tag="p8")
                        ts = [psc.tile([P, 2, 512], F32,
                                       name=f"sc{b}{mc}{j}{hh}{u}", tag="sc")
                              for u in range(2)]
                        for i in range(2):
                            for u in range(2):
                                lc = 2 * u + i
                                nc.tensor.matmul(
                                    ts[u][:, i, :],
                                    st["k8"][:, h, lc * P:(lc + 1) * P],
                                    st["q8"][:, h, mc, :],
                                    start=True, stop=True)
                        for u in range(2):
                            nc.scalar.activation(
                                p8[:, u, :, :], ts[u][:],
                                mybir.ActivationFunctionType.Exp,
                                scale=1.0 / 64.0)
                        ps_h.append(p8)
                    return ps_h

                def attnv_group(mc, j, ps_h):
                    for hh in range(2):
                        h = 2 * j + hh
                        aug = paug.tile([96, 512], F32,
                                        name=f"aug{b}{mc}{j}{hh}", tag="aug")
                        for u in range(2):
                            nc.tensor.matmul(aug[:], st["v8"][:, u, :, h, :],
                                             ps_h[hh][:, u, :, :],
                                             start=(u == 0), stop=(u == 1),
                                             perf_mode=DR)
                        asb = ppool.tile([HD + 1, 512], F32,
                                         name=f"asb{b}{mc}{j}{hh}", tag="asb",
                                         bufs=12)
                        nc.vector.tensor_copy(asb[:], aug[0:HD + 1, :])
                        nc.sync.dma_start(st["den"][mc][h:h + 1, :],
                                          asb[HD:HD + 1, :])
                        st["asb"][mc][h] = asb

                def norm(mc):
                    rcpf = spool.tile([NH, 512], F32, name=f"rcpf{b}{mc}", tag="rcpf")
                    nc.vector.reciprocal_approx_fast(rcpf[:], st["den"][mc][:])
                    rcp = spool.tile([NH, 512], BF16, name=f"rcp{b}{mc}", tag="rcp")
                    nc.vector.tensor_copy(rcp[:], rcpf[:])
                    rcp_d = dpool.tile([NH, 512], BF16, name=f"rcpd{b}{mc}", tag="rcpd")
                    nc.sync.dma_start(rcp_d[:], rcp[:])
                    for h in range(NH):
                        j, hh = h // 2, h % 2
                        rcb = spool.tile([HD, 512], BF16,
                                         name=f"rcb{b}{mc}{h}", tag="rcb",
                                         bufs=6)
                        # alternate queues: 8 serial broadcasts otherwise gate
                        # the normalize tail
                        dq = nc.sync.dma_start if hh == 0 else nc.scalar.dma_start
                        dq(rcb[:], rcp_d[h:h + 1, :].to_broadcast((HD, 512)))
                        # split normalize between DVE and the idle gpsimd
                        eng = nc.vector if hh == 0 else nc.gpsimd
                        with nc.allow_low_precision(reason="fp8 attn operand; error damped by residual"):
                            eng.tensor_tensor(
                                st["an8"][hh * HD:(hh + 1) * HD, j, mc, :],
                                st["asb"][mc][h][0:HD, :], rcb[:],
                                op=mybir.AluOpType.mult)

                def outproj(cc, mc):
                    ms = slice(mc * 512, (mc + 1) * 512)
                    cs = slice(cc * P, (cc + 1) * P)
                    op = psum.tile([P, 512], F32, name=f"op{b}{cc}{mc}", tag="ps")
                    for u in range(CC // 2):
                        nc.tensor.matmul(op[:], wo_sb[:, 2 * u:2 * u + 2, cs],
                                         st["an8"][:, 2 * u:2 * u + 2, mc, :],
                                         start=(u == 0), stop=(u == CC // 2 - 1),
                                         perf_mode=DR)
                    y_sb = xpool.tile([P, 512], F32, name=f"y{b}{cc}{mc}",
                                      tag="y", bufs=3)
                    nc.vector.scalar_tensor_tensor(
                        y_sb[:], op[:], 1.0 / 4096.0, st["xbf"][:, cc, ms],
                        op0=mybir.AluOpType.mult, op1=mybir.AluOpType.add)
                    # outputs ride the gpsimd queue so the final writes don't
                    # drain behind the sync queue's normalize broadcasts
                    nc.gpsimd.dma_start(
                        yd.ap()[b][cc * P:(cc + 1) * P, mc * 512:(mc + 1) * 512],
                        y_sb[:])

                return dict(loads=loads, stats=stats, k_chain=k_chain,
                            v_chain=v_chain, q_chain=q_chain,
                            sc_exp_group=sc_exp_group, attnv_group=attnv_group,
                            norm=norm, outproj=outproj)

            # ---- software-pipelined emission (cross-batch modulo schedule) ----
            # PE fillers sit between ACT-bound score/exp groups and their
            # attn@v consumers; fillers are chosen to be independent of the
            # preceding normalize latency.
            E = [make_batch(b) for b in range(BPC)]

            def attn_pass(eb, mc, fillers, post_first=None):
                # attn@v trails the score/exp groups by one j so its P tiles
                # (ACT exps) are complete; fillers keep PE fed in between.
                prev = None
                for j in range(NH // 2):
                    ps_h = eb["sc_exp_group"](mc, j)
                    fillers[j]()
                    if prev is not None:
                        eb["attnv_group"](mc, j - 1, prev)
                        if j == 1 and post_first is not None:
                            post_first()
                    prev = ps_h
                eb["attnv_group"](mc, NH // 2 - 1, prev)

            def nop():
                pass

            E[0]["loads"]()
            E[0]["stats"](0)
            E[0]["stats"](1)
            for lc in range(LC):
                E[0]["v_chain"](lc)
            E[0]["k_chain"](0)
            E[0]["q_chain"](0, 0)
            E[0]["q_chain"](0, 1)

            def kq(eb, j):
                def f():
                    eb["k_chain"](j)
                    eb["q_chain"](j, 0)
                    eb["q_chain"](j, 1)
                return f

            attn_pass(E[0], 0, [kq(E[0], 1), kq(E[0], 2),
                                lambda: (E[1]["loads"](), kq(E[0], 3)()),
                                nop])
            attn_pass(E[0], 1,
                      [lambda: (E[1]["stats"](0), E[1]["stats"](1)),
                       lambda: (E[1]["v_chain"](0), E[1]["v_chain"](1)),
                       lambda: (E[1]["v_chain"](2), E[1]["v_chain"](3)),
                       kq(E[1], 0)],
                      post_first=lambda: E[0]["norm"](0))
            attn_pass(E[1], 0,
                      [lambda: (E[0]["outproj"](0, 0), E[1]["k_chain"](1),
                                E[1]["q_chain"](1, 0), E[1]["q_chain"](1, 1)),
                       lambda: (E[0]["outproj"](1, 0), E[1]["k_chain"](2),
                                E[1]["q_chain"](2, 0), E[1]["q_chain"](2, 1)),
                       lambda: (E[0]["outproj"](2, 0), E[1]["k_chain"](3),
                                E[1]["q_chain"](3, 0), E[1]["q_chain"](3, 1)),
                       lambda: E[0]["outproj"](3, 0)],
                      post_first=lambda: E[0]["norm"](1))
            attn_pass(E[1], 1,
                      [lambda: E[0]["outproj"](0, 1),
                       lambda: E[0]["outproj"](1, 1),
                       lambda: (E[0]["outproj"](2, 1), E[1]["outproj"](0, 0)),
                       lambda: (E[0]["outproj"](3, 1), E[1]["outproj"](1, 0))],
                      post_first=lambda: E[1]["norm"](0))
            E[1]["outproj"](2, 0)
            E[1]["outproj"](3, 0)
            E[1]["norm"](1)
            for cc in range(CC):
                E[1]["outproj"](cc, 1)
    nc.compile()
    return nc


def _get_nc():
    global _NC_CACHE
    if _NC_CACHE is None:
        _NC_CACHE = _build()
    return _NC_CACHE


def kernel(x, context, context_mask, ln_w, ln_b, Wq, Wk, Wv, Wo, Wctx):
    x = np.asarray(x, np.float32)
    context = np.asarray(context, np.float32)
    context_mask = np.asarray(context_mask)
    ln_w = np.asarray(ln_w, np.float32)
    ln_b = np.asarray(ln_b, np.float32)
    Wq = np.asarray(Wq, np.float32)
    Wk = np.asarray(Wk, np.float32)
    Wv = np.asarray(Wv, np.float32)
    Wo = np.asarray(Wo, np.float32)
    Wctx = np.asarray(Wctx, np.float32)

    scale = HD ** -0.5
    # interleave the two heads of each 128-wide e-chunk (row r -> head r%2,
    # dim r//2) so one [128,512] DMA reshuffles q/k into per-head layout
    perm = np.arange(C).reshape(CC, 2, HD).transpose(0, 2, 1).reshape(C)
    wq_f = Wq * (ln_w[None, :] * scale)          # [E, C] ln scale + attn scale
    wq8 = np.ascontiguousarray(wq_f.T[:, perm] * 64.0).astype(F8)
    q_r2 = np.stack([64.0 * wq_f.sum(1)[perm],
                     1024.0 * ((Wq * scale) @ ln_b)[perm]]).astype(BF)  # [2, E]
    wck8 = np.ascontiguousarray((Wk @ Wctx).T[:, perm] * 16.0).astype(F8)
    wcv8 = np.ascontiguousarray((Wv @ Wctx).T * 16.0).astype(F8)    # [768, 512]
    wo8 = np.ascontiguousarray(Wo.T * 32.0).astype(F8)

    xr = x.reshape(NCORES, BPC, C, N)
    x8 = xr.astype(F8)
    xbf = xr.astype(BF)
    ctx8 = np.ascontiguousarray(
        context.transpose(0, 2, 1)).astype(F8).reshape(NCORES, BPC, CTX_DIM, L)
    # k8 aug row 64: -224 on masked keys; paired with the constant-16 q aug
    # row and the 1/64 exp descale it contributes -56 -> exp() == 0
    mrow8 = np.broadcast_to(
        ((~context_mask).astype(np.float32) * -224.0)[:, None, None, :],
        (B, 1, NH, L)).astype(F8).reshape(NCORES, BPC, 1, NH, L)
    qaug8 = np.full((1, NH, MC, 512), 16.0, np.float32).astype(F8)

    in_maps = [
        {"x8": np.ascontiguousarray(x8[c]), "xbf": np.ascontiguousarray(xbf[c]),
         "ctx8": np.ascontiguousarray(ctx8[c]),
         "mrow8": np.ascontiguousarray(mrow8[c]), "qaug8": qaug8,
         "wq8": wq8, "wck8": wck8,
         "wcv8": wcv8, "wo8": wo8, "q_r2": q_r2}
        for c in range(NCORES)
    ]
    res = run_bass_kernel_spmd(_get_nc(), in_maps, core_ids=list(range(NCORES)))
    y = np.stack([r["y"] for r in res.results])          # [8, 2, C, N]
    return y.reshape(B, C, H, W)


# revision 56
# speedup vs baseline: 1.1009x; 1.1009x over previous
"""ContextualAttention2D Trainium2 kernel.

Full inputs -> full output; internally data-parallel over batch across 8
NeuronCores (2 batches per core), single SPMD NEFF, no collectives.

Math (per batch):
  hidden[n,c]   = x.reshape(C, H*W).T
  hn            = layernorm_c(hidden) * ln_w + ln_b
  q             = hn @ Wq.T ;  k = ctx @ Wk.T ; v = ctx @ Wv.T
  ctx           = context @ Wctx.T      (folded: k = context @ (Wk@Wctx).T etc)
  attn          = softmax_l(q @ k.T * hd^-0.5 + maskbias) ; out = attn @ v
  y             = (out @ Wo.T + hidden).T.reshape(C, H, W)

fp8 (e4m3) DoubleRow matmuls carry the projection GEMMs and attn@V at
0.5 cycles/row with two 128-deep k-tiles per instruction.  Per-tensor
power-of-two scales keep every fp8 operand in range:
  wq8 = 64*Wq*ln_w/8          q_psum = 64*q    rbc = rstd/1024 (Rsqrt scale)
  wck8 = 16*(Wk@Wctx).T       k_sb  = 16*k     (1/16 cancelled by rbc)
  wcv8 = 16*(Wv@Wctx).T       v8    = 16*v     aug ones col = 1/8
  probs = exp(scores+mb) e4m3 den8 = den/8     rcb2 = 8/den  -> an8 = 128*attn
  wo8 = 32*Wo.T               out_psum = 4096*out, residual add scales 1/4096

LayerNorm: per-token mean/var via ones-matmuls (cross-partition sum);
rstd/1024 from ACT Rsqrt (scale=2^20), invr via DVE (var+eps)*rstd; the
mean correction enters Q as a rank-2 bf16 matmul into the same PSUM group.

Softmax denominator: attn@V is augmented with a 1/8 ones column; the
denominator row is DMA-gathered into a [128,32] tile so one DVE
reciprocal covers all 8 heads, then DMA-broadcast back per head-pair.
attn@V PSUM is evicted by DMA (off-engine) and normalized straight into
the fp8 out-projection operand.
"""
import numpy as np
import ml_dtypes

from concourse import bacc, mybir, tile
from concourse.bass_utils import run_bass_kernel_spmd

BF = ml_dtypes.bfloat16
F8 = ml_dtypes.float8_e4m3

B, C, H, W = 16, 512, 32, 32
NH, HD = 8, 64
CTX_DIM, L = 768, 512
EPS = 1e-5
N = H * W                 # 1024 tokens
NCORES = 8
BPC = B // NCORES         # batches per core
P = 128
CC = C // P               # 4 c-chunks
DC = CTX_DIM // P         # 6 d-chunks
LC = L // P               # 4 l-chunks
MC = N // 512             # 2 token chunks of 512
MASK_NEG = -30000.0

F32 = mybir.dt.float32
BF16 = mybir.dt.bfloat16
FP8 = mybir.dt.float8e4
DR = mybir.MatmulPerfMode.DoubleRow

_NC_CACHE = None


def _build():
    nc = bacc.Bacc(None, target_bir_lowering=False, debug=False)

    x8d = nc.dram_tensor("x8", [BPC, C, N], FP8, kind="ExternalInput")
    xbfd = nc.dram_tensor("xbf", [BPC, C, N], BF16, kind="ExternalInput")
    ctx8d = nc.dram_tensor("ctx8", [BPC, CTX_DIM, L], FP8, kind="ExternalInput")
    # fp8 aug rows: mask row for k8 (i=0: -224 masked / 0, i=1: zeros) and
    # the constant 16 / 0 rows for q8
    mrowd = nc.dram_tensor("mrow8", [BPC, 1, NH, L], FP8, kind="ExternalInput")
    qaugd = nc.dram_tensor("qaug8", [1, NH, MC, 512], FP8, kind="ExternalInput")
    wq8d = nc.dram_tensor("wq8", [C, C], FP8, kind="ExternalInput")
    wck8d = nc.dram_tensor("wck8", [CTX_DIM, C], FP8, kind="ExternalInput")
    wcv8d = nc.dram_tensor("wcv8", [CTX_DIM, C], FP8, kind="ExternalInput")
    wo8d = nc.dram_tensor("wo8", [C, C], FP8, kind="ExternalInput")
    qr2d = nc.dram_tensor("q_r2", [2, C], BF16, kind="ExternalInput")
    yd = nc.dram_tensor("y", [BPC, C, N], F32, kind="ExternalOutput")

    with tile.TileContext(nc) as tc:
        with (
            tc.tile_pool(name="wpool", bufs=1) as wpool,
            tc.tile_pool(name="xpool", bufs=2) as xpool,
            tc.tile_pool(name="actpool", bufs=2) as actpool,
            tc.tile_pool(name="ppool", bufs=6) as ppool,
            tc.tile_pool(name="spool", bufs=2) as spool,
            tc.tile_pool(name="psum", bufs=2, space="PSUM") as psum,
            tc.tile_pool(name="psc", bufs=2, space="PSUM") as psc,
            tc.tile_pool(name="paug", bufs=2, space="PSUM") as paug,
            tc.tile_pool(name="dpool", bufs=4, space="DRAM") as dpool,
        ):
            # ---- persistent weights ----
            wq_sb = wpool.tile([P, CC, C], FP8)
            nc.scalar.dma_start(wq_sb[:], wq8d.ap().rearrange("(cc p) e -> p cc e", p=P))
            wck_sb = wpool.tile([P, DC, C], FP8)
            nc.scalar.dma_start(wck_sb[:], wck8d.ap().rearrange("(dc p) e -> p dc e", p=P))
            wcv_sb = wpool.tile([P, DC, C], FP8)
            nc.scalar.dma_start(wcv_sb[:], wcv8d.ap().rearrange("(dc p) e -> p dc e", p=P))
            wo_sb = wpool.tile([P, CC, C], FP8)
            nc.scalar.dma_start(wo_sb[:], wo8d.ap().rearrange("(ec p) c -> p ec c", p=P))
            qr2_sb = wpool.tile([2, C], BF16)
            nc.scalar.dma_start(qr2_sb[:], qr2d.ap())

            ones1_sb = wpool.tile([P, 1], BF16)   # stats lhsT (column sums)
            nc.vector.memset(ones1_sb[:], 1.0)
            onesr_sb = wpool.tile([1, P], BF16)    # bcast-matmul lhsT (rank-1)
            nc.vector.memset(onesr_sb[:], 1.0)
            eps2_sb = wpool.tile([1, 1], F32)      # eps * 2^8 (scaled Sqrt bias)
            nc.vector.memset(eps2_sb[:], EPS * 256.0)

            # Per-batch emission closures; emitted in a software-pipelined
            # order so PE filler (projection chains) sits between the
            # ACT-bound score-exp groups and their attn@v consumers.
            def make_batch(b):
                st = {}

                def loads():
                    # spread bulk loads across queues so ctx (feeds v/k), xbf
                    # (feeds stats) and x8 (feeds q) all stream in parallel;
                    # b0's ctx rides sync, later batches keep off sync so the
                    # previous batch's latency-sensitive normalize DMAs win.
                    bulk = nc.sync.dma_start if b == 0 else nc.gpsimd.dma_start
                    st["x8"] = xpool.tile([P, CC, N], FP8, name=f"x8{b}", tag="x8")
                    st["xbf"] = xpool.tile([P, CC, N], BF16, name=f"xbf{b}", tag="xbf")
                    st["ctx8"] = xpool.tile([P, DC, L], FP8, name=f"ctx8{b}", tag="ctx8")
                    for dc in range(DC):
                        bulk(st["ctx8"][:, dc, :],
                             ctx8d.ap()[b][dc * P:(dc + 1) * P, :])
                    for cc in range(CC):
                        nc.gpsimd.dma_start(st["xbf"][:, cc, :],
                                            xbfd.ap()[b][cc * P:(cc + 1) * P, :])
                    for cc in range(CC):
                        nc.scalar.dma_start(
                            st["x8"][:, cc, :],
                            x8d.ap()[b][cc * P:(cc + 1) * P, :])
                    # scores operands: [65, h, ...] fp8, aug row 64 carries the
                    # mask on the k side and a constant 16 on the q side.
                    # Head dims are host-interleaved so one [128,512] DMA
                    # fills both heads of an ec chunk (row r -> head r%2).
                    st["k8"] = actpool.tile([65, NH, L], FP8,
                                            name=f"k8{b}", tag="k8")
                    st["q8"] = actpool.tile([65, NH, MC, 512], FP8,
                                            name=f"q8{b}", tag="q8")
                    nc.sync.dma_start(st["k8"][64:65, :, :], mrowd.ap()[b])
                    nc.sync.dma_start(st["q8"][64:65, :, :, :], qaugd.ap()[0])
                    st["xsq"] = xpool.tile([P, CC, N], BF16, name=f"xsq{b}",
                                           tag="xsq", bufs=1)
                    for cc in range(CC):
                        nc.gpsimd.tensor_tensor(
                            st["xsq"][:, cc, :], st["xbf"][:, cc, :],
                            st["xbf"][:, cc, :], op=mybir.AluOpType.mult)

                    # v8: [d, lc-pair u, k-tile i, head, 96]; col 64 = 1/8 ones
                    # (denominator), cols 65:96 zero pad (DoubleRow stationary
                    # width must be a multiple of 32)
                    st["v8"] = actpool.tile([P, LC // 2, 2, NH, 96], FP8,
                                            name=f"v8{b}", tag="v8")
                    nc.vector.memset(st["v8"][:, :, :, :, HD + 1:], 0.0)
                    nc.vector.memset(st["v8"][:, :, :, :, HD:HD + 1], 0.125)
                    st["an8"] = actpool.tile([P, CC, MC, 512], FP8,
                                             name=f"an8{b}", tag="an8")
                    st["r2"] = {}
                    st["rbc"] = {}
                    st["den"] = {}
                    st["asb"] = {}
                    st["rcb"] = {}

                def stats(mc):
                    ms = slice(mc * 512, (mc + 1) * 512)
                    st1 = psum.tile([1, 512], F32, name=f"st1{b}{mc}", tag="ps")
                    for cc in range(CC):
                        nc.tensor.matmul(st1[:], ones1_sb[:], st["xbf"][:, cc, ms],
                                         start=(cc == 0), stop=(cc == CC - 1))
                    st2 = psum.tile([1, 512], F32, name=f"st2{b}{mc}", tag="ps")
                    for cc in range(CC):
                        nc.tensor.matmul(st2[:], ones1_sb[:], st["xsq"][:, cc, ms],
                                         start=(cc == 0), stop=(cc == CC - 1))
                    negmu = spool.tile([1, 512], BF16, name=f"negmu{b}{mc}", tag="negmu")
                    nc.vector.tensor_scalar_mul(negmu[:], st1[:], -1.0 / C)
                    musq = spool.tile([1, 512], F32, name=f"musq{b}{mc}", tag="musq")
                    nc.vector.tensor_tensor(musq[:], negmu[:], negmu[:],
                                            op=mybir.AluOpType.mult)
                    var = spool.tile([1, 512], F32, name=f"var{b}{mc}", tag="var")
                    nc.vector.scalar_tensor_tensor(
                        var[:], st2[:], 1.0 / C, musq[:],
                        op0=mybir.AluOpType.mult, op1=mybir.AluOpType.subtract)
                    # invr_k = sqrt((var+eps)*2^8) = 16*invr  (fp32 for recip)
                    invr_k = spool.tile([1, 512], F32, name=f"invk{b}{mc}", tag="invk")
                    nc.scalar.activation(invr_k[:], var[:],
                                         mybir.ActivationFunctionType.Sqrt,
                                         bias=eps2_sb[:], scale=256.0)
                    rstd_f = spool.tile([1, 512], F32, name=f"rstf{b}{mc}", tag="rstf")
                    nc.vector.reciprocal_approx_fast(rstd_f[:], invr_k[:])
                    rstd = spool.tile([1, 512], BF16, name=f"rstd{b}{mc}", tag="rstd")
                    nc.vector.tensor_copy(rstd[:], rstd_f[:])
                    # invr/16 = (var+eps) * (rstd/16)
                    invr = spool.tile([1, 512], BF16, name=f"invr{b}{mc}", tag="invr")
                    nc.vector.scalar_tensor_tensor(
                        invr[:], var[:], EPS, rstd[:],
                        op0=mybir.AluOpType.add, op1=mybir.AluOpType.mult)
                    r2 = spool.tile([2, 512], BF16, name=f"r2_{b}{mc}", tag="r2")
                    nc.sync.dma_start(r2[0:1, :], negmu[:])
                    nc.sync.dma_start(r2[1:2, :], invr[:])
                    rbp = paug.tile([P, 512], F32, name=f"rbp{b}{mc}", tag="aug")
                    nc.tensor.matmul(rbp[:], onesr_sb[:], rstd[:],
                                     start=True, stop=True)
                    rbc = spool.tile([P, 512], BF16, name=f"rbc{b}{mc}", tag="rbc")
                    nc.vector.tensor_copy(rbc[:], rbp[:])
                    st["r2"][mc] = r2
                    st["rbc"][mc] = rbc

                def k_chain(ec):
                    es = slice(ec * P, (ec + 1) * P)
                    kp = psum.tile([P, 512], F32, name=f"kp{b}{ec}", tag="ps")
                    for u in range(DC // 2):
                        nc.tensor.matmul(kp[:], wck_sb[:, 2 * u:2 * u + 2, es],
                                         st["ctx8"][:, 2 * u:2 * u + 2, :],
                                         start=(u == 0), stop=(u == DC // 2 - 1),
                                         perf_mode=DR)
                    ktmp = ppool.tile([P, 512], FP8, name=f"ktmp{b}{ec}",
                                      tag="ktmp", bufs=3)
                    with nc.allow_low_precision(reason="fp8 attn operand; error damped by residual"):
                        nc.vector.tensor_copy(ktmp[:], kp[:])
                    # partition reshuffle rides the DMA engines
                    # (k8[p, 2ec + r%2] = ktmp[2p + r%2] via host interleave)
                    nc.gpsimd.dma_start(
                        st["k8"][0:HD, 2 * ec:2 * ec + 2, :], ktmp[:])

                def v_chain(lc):
                    ls = slice(lc * P, (lc + 1) * P)
                    vp = psum.tile([P, 512], F32, name=f"vp{b}{lc}", tag="ps")
                    for u in range(DC // 2):
                        nc.tensor.matmul(vp[:], st["ctx8"][:, 2 * u:2 * u + 2, ls],
                                         wcv_sb[:, 2 * u:2 * u + 2, :],
                                         start=(u == 0), stop=(u == DC // 2 - 1),
                                         perf_mode=DR)
                    with nc.allow_low_precision(reason="fp8 attn values; error damped by residual"):
                        nc.vector.tensor_copy(
                            st["v8"][:, lc // 2, lc % 2, :, 0:HD],
                            vp[:].rearrange("p (h d) -> p h d", d=HD))

                def q_chain(ec, mc):
                    es = slice(ec * P, (ec + 1) * P)
                    ms = slice(mc * 512, (mc + 1) * 512)
                    qp = psum.tile([P, 512], F32, name=f"qp{b}{ec}{mc}", tag="ps")
                    for u in range(CC // 2):
                        nc.tensor.matmul(qp[:], wq_sb[:, 2 * u:2 * u + 2, es],
                                         st["x8"][:, 2 * u:2 * u + 2, ms],
                                         start=(u == 0), stop=False,
                                         perf_mode=DR)
                    nc.tensor.matmul(qp[:], qr2_sb[:, es], st["r2"][mc][:],
                                     start=False, stop=True)
                    qtmp = ppool.tile([P, 512], FP8, name=f"qtmp{b}{ec}{mc}",
                                      tag="qtmp", bufs=3)
                    with nc.allow_low_precision(reason="fp8 attn operand; error damped by residual"):
                        nc.vector.tensor_tensor(qtmp[:], qp[:],
                                                st["rbc"][mc][:],
                                                op=mybir.AluOpType.mult)
                    nc.gpsimd.dma_start(
                        st["q8"][0:HD, 2 * ec:2 * ec + 2, mc, :], qtmp[:])

                def sc_exp_group(mc, j):
                    if mc not in st["den"]:
                        st["den"][mc] = spool.tile([NH, 512], F32,
                                                   name=f"den{b}{mc}", tag="den")
                        st["asb"][mc] = {}
                    # probs for this head-pair: per hh a [P, 2(u: lc pair),
                    # 2(i), 512] fp8 tile feeding the DoubleRow attn@v.
                    # scores are DoubleRow too ([33,2,*] k/q with the mask and
                    # a 16-const in aug row 32); exp applies the 1/64 descale.
                    ps_h = []
                    for hh in range(2):
                        h = 2 * j + hh
                        p8 = ppool.tile([P, 2, 2, 512], FP8,
                                        name=f"p8{b}{mc}{j}{hh}", tag="p8")
                        ts = [psc.tile([P, 2, 512], F32,
                                       name=f"sc{b}{mc}{j}{hh}{u}", tag="sc")
                              for u in range(2)]
                        for i in range(2):
                            for u in range(2):
                                lc = 2 * u + i
                                nc.tensor.matmul(
                                    ts[u][:, i, :],
                                    st["k8"][:, h, lc * P:(lc + 1) * P],
                                    st["q8"][:, h, mc, :],
                                    start=True, stop=True)
                        for u in range(2):
                            nc.scalar.activation(
                                p8[:, u, :, :], ts[u][:],
                                mybir.ActivationFunctionType.Exp,
                                scale=1.0 / 64.0)
                        ps_h.append(p8)
                    return ps_h

                def attnv_group(mc, j, ps_h):
                    for hh in range(2):
                        h = 2 * j + hh
                        aug = paug.tile([96, 512], F32,
                                        name=f"aug{b}{mc}{j}{hh}", tag="aug")
                        for u in range(2):
                            nc.tensor.matmul(aug[:], st["v8"][:, u, :, h, :],
                                             ps_h[hh][:, u, :, :],
                                             start=(u == 0), stop=(u == 1),
                                             perf_mode=DR)
                        asb = ppool.tile([HD + 1, 512], F32,
                                         name=f"asb{b}{mc}{j}{hh}", tag="asb",
                                         bufs=12)
                        nc.vector.tensor_copy(asb[:], aug[0:HD + 1, :])
                        nc.sync.dma_start(st["den"][mc][h:h + 1, :],
                                          asb[HD:HD + 1, :])
                        st["asb"][mc][h] = asb

                def norm(mc):
                    rcpf = spool.tile([NH, 512], F32, name=f"rcpf{b}{mc}", tag="rcpf")
                    nc.vector.reciprocal_approx_fast(rcpf[:], st["den"][mc][:])
                    rcp = spool.tile([NH, 512], BF16, name=f"rcp{b}{mc}", tag="rcp")
                    nc.vector.tensor_copy(rcp[:], rcpf[:])
                    rcp_d = dpool.tile([NH, 512], BF16, name=f"rcpd{b}{mc}", tag="rcpd")
                    nc.sync.dma_start(rcp_d[:], rcp[:])
                    for h in range(NH):
                        j, hh = h // 2, h % 2
                        rcb = spool.tile([HD, 512], BF16,
                                         name=f"rcb{b}{mc}{h}", tag="rcb",
                                         bufs=6)
                        nc.sync.dma_start(
                            rcb[:], rcp_d[h:h + 1, :].to_broadcast((HD, 512)))
                        # split normalize between DVE and the idle gpsimd
                        eng = nc.vector if hh == 0 else nc.gpsimd
                        with nc.allow_low_precision(reason="fp8 attn operand; error damped by residual"):
                            eng.tensor_tensor(
                                st["an8"][hh * HD:(hh + 1) * HD, j, mc, :],
                                st["asb"][mc][h][0:HD, :], rcb[:],
                                op=mybir.AluOpType.mult)

                def outproj(cc, mc):
                    ms = slice(mc * 512, (mc + 1) * 512)
                    cs = slice(cc * P, (cc + 1) * P)
                    op = psum.tile([P, 512], F32, name=f"op{b}{cc}{mc}", tag="ps")
                    for u in range(CC // 2):
                        nc.tensor.matmul(op[:], wo_sb[:, 2 * u:2 * u + 2, cs],
                                         st["an8"][:, 2 * u:2 * u + 2, mc, :],
                                         start=(u == 0), stop=(u == CC // 2 - 1),
                                         perf_mode=DR)
                    y_sb = xpool.tile([P, 512], F32, name=f"y{b}{cc}{mc}",
                                      tag="y", bufs=3)
                    nc.vector.scalar_tensor_tensor(
                        y_sb[:], op[:], 1.0 / 4096.0, st["xbf"][:, cc, ms],
                        op0=mybir.AluOpType.mult, op1=mybir.AluOpType.add)
                    nc.sync.dma_start(
                        yd.ap()[b][cc * P:(cc + 1) * P, mc * 512:(mc + 1) * 512],
                        y_sb[:])

                return dict(loads=loads, stats=stats, k_chain=k_chain,
                            v_chain=v_chain, q_chain=q_chain,
                            sc_exp_group=sc_exp_group, attnv_group=attnv_group,
                            norm=norm, outproj=outproj)

            # ---- software-pipelined emission (cross-batch modulo schedule) ----
            # PE fillers sit between ACT-bound score/exp groups and their
            # attn@v consumers; fillers are chosen to be independent of the
            # preceding normalize latency.
            E = [make_batch(b) for b in range(BPC)]

            def attn_pass(eb, mc, fillers, post_first=None):
                # attn@v trails the score/exp groups by one j so its P tiles
                # (ACT exps) are complete; fillers keep PE fed in between.
                prev = None
                for j in range(NH // 2):
                    ps_h = eb["sc_exp_group"](mc, j)
                    fillers[j]()
                    if prev is not None:
                        eb["attnv_group"](mc, j - 1, prev)
                        if j == 1 and post_first is not None:
                            post_first()
                    prev = ps_h
                eb["attnv_group"](mc, NH // 2 - 1, prev)

            def nop():
                pass

            E[0]["loads"]()
            E[0]["stats"](0)
            E[0]["stats"](1)
            for lc in range(LC):
                E[0]["v_chain"](lc)
            E[0]["k_chain"](0)
            E[0]["q_chain"](0, 0)
            E[0]["q_chain"](0, 1)

            def kq(eb, j):
                def f():
                    eb["k_chain"](j)
                    eb["q_chain"](j, 0)
                    eb["q_chain"](j, 1)
                return f

            attn_pass(E[0], 0, [kq(E[0], 1), kq(E[0], 2),
                                lambda: (E[1]["loads"](), kq(E[0], 3)()),
                                nop])
            attn_pass(E[0], 1,
                      [lambda: (E[1]["stats"](0), E[1]["stats"](1)),
                       lambda: (E[1]["v_chain"](0), E[1]["v_chain"](1)),
                       lambda: (E[1]["v_chain"](2), E[1]["v_chain"](3)),
                       kq(E[1], 0)],
                      post_first=lambda: E[0]["norm"](0))
            attn_pass(E[1], 0,
                      [lambda: (E[0]["outproj"](0, 0), E[1]["k_chain"](1),
                                E[1]["q_chain"](1, 0), E[1]["q_chain"](1, 1)),
                       lambda: (E[0]["outproj"](1, 0), E[1]["k_chain"](2),
                                E[1]["q_chain"](2, 0), E[1]["q_chain"](2, 1)),
                       lambda: (E[0]["outproj"](2, 0), E[1]["k_chain"](3),
                                E[1]["q_chain"](3, 0), E[1]["q_chain"](3, 1)),
                       lambda: E[0]["outproj"](3, 0)],
                      post_first=lambda: E[0]["norm"](1))
            attn_pass(E[1], 1,
                      [lambda: E[0]["outproj"](0, 1),
                       lambda: E[0]["outproj"](1, 1),
                       lambda: (E[0]["outproj"](2, 1), E[1]["outproj"](0, 0)),
                       lambda: (E[0]["outproj"](3, 1), E[1]["outproj"](1, 0))],
                      post_first=lambda: E[1]["norm"](0))
            E[1]["outproj"](2, 0)
            E[1]["outproj"](3, 0)
            E[1]["norm"](1)
            for cc in range(CC):
                E[1]["outproj"](cc, 1)
    nc.compile()
    return nc


def _get_nc():
    global _NC_CACHE
    if _NC_CACHE is None:
        _NC_CACHE = _build()
    return _NC_CACHE


def kernel(x, context, context_mask, ln_w, ln_b, Wq, Wk, Wv, Wo, Wctx):
    x = np.asarray(x, np.float32)
    context = np.asarray(context, np.float32)
    context_mask = np.asarray(context_mask)
    ln_w = np.asarray(ln_w, np.float32)
    ln_b = np.asarray(ln_b, np.float32)
    Wq = np.asarray(Wq, np.float32)
    Wk = np.asarray(Wk, np.float32)
    Wv = np.asarray(Wv, np.float32)
    Wo = np.asarray(Wo, np.float32)
    Wctx = np.asarray(Wctx, np.float32)

    scale = HD ** -0.5
    # interleave the two heads of each 128-wide e-chunk (row r -> head r%2,
    # dim r//2) so one [128,512] DMA reshuffles q/k into per-head layout
    perm = np.arange(C).reshape(CC, 2, HD).transpose(0, 2, 1).reshape(C)
    wq_f = Wq * (ln_w[None, :] * scale)          # [E, C] ln scale + attn scale
    wq8 = np.ascontiguousarray(wq_f.T[:, perm] * 64.0).astype(F8)
    q_r2 = np.stack([64.0 * wq_f.sum(1)[perm],
                     1024.0 * ((Wq * scale) @ ln_b)[perm]]).astype(BF)  # [2, E]
    wck8 = np.ascontiguousarray((Wk @ Wctx).T[:, perm] * 16.0).astype(F8)
    wcv8 = np.ascontiguousarray((Wv @ Wctx).T * 16.0).astype(F8)    # [768, 512]
    wo8 = np.ascontiguousarray(Wo.T * 32.0).astype(F8)

    xr = x.reshape(NCORES, BPC, C, N)
    x8 = xr.astype(F8)
    xbf = xr.astype(BF)
    ctx8 = np.ascontiguousarray(
        context.transpose(0, 2, 1)).astype(F8).reshape(NCORES, BPC, CTX_DIM, L)
    # k8 aug row 64: -224 on masked keys; paired with the constant-16 q aug
    # row and the 1/64 exp descale it contributes -56 -> exp() == 0
    mrow8 = np.broadcast_to(
        ((~context_mask).astype(np.float32) * -224.0)[:, None, None, :],
        (B, 1, NH, L)).astype(F8).reshape(NCORES, BPC, 1, NH, L)
    qaug8 = np.full((1, NH, MC, 512), 16.0, np.float32).astype(F8)

    in_maps = [
        {"x8": np.ascontiguousarray(x8[c]), "xbf": np.ascontiguousarray(xbf[c]),
         "ctx8": np.ascontiguousarray(ctx8[c]),
         "mrow8": np.ascontiguousarray(mrow8[c]), "qaug8": qaug8,
         "wq8": wq8, "wck8": wck8,
         "wcv8": wcv8, "wo8": wo8, "q_r2": q_r2}
        for c in range(NCORES)
    ]
    res = run_bass_kernel_spmd(_get_nc(), in_maps, core_ids=list(range(NCORES)))
    y = np.stack([r["y"] for r in res.results])          # [8, 2, C, N]
    return y.reshape(B, C, H, W)


# revision 64
# speedup vs baseline: 1.1100x; 1.0083x over previous
"""ContextualAttention2D Trainium2 kernel.

Full inputs -> full output; internally data-parallel over batch across 8
NeuronCores (2 batches per core), single SPMD NEFF, no collectives.

Math (per batch):
  hidden[n,c]   = x.reshape(C, H*W).T
  hn            = layernorm_c(hidden) * ln_w + ln_b
  q             = hn @ Wq.T ;  k = ctx @ Wk.T ; v = ctx @ Wv.T
  ctx           = context @ Wctx.T      (folded: k = context @ (Wk@Wctx).T etc)
  attn          = softmax_l(q @ k.T * hd^-0.5 + maskbias) ; out = attn @ v
  y             = (out @ Wo.T + hidden).T.reshape(C, H, W)

fp8 (e4m3) DoubleRow matmuls carry the projection GEMMs and attn@V at
0.5 cycles/row with two 128-deep k-tiles per instruction.  Per-tensor
power-of-two scales keep every fp8 operand in range:
  wq8 = 64*Wq*ln_w/8          q_psum = 64*q    rbc = rstd/1024 (Rsqrt scale)
  wck8 = 16*(Wk@Wctx).T       k_sb  = 16*k     (1/16 cancelled by rbc)
  wcv8 = 16*(Wv@Wctx).T       v8    = 16*v     aug ones col = 1/8
  probs = exp(scores+mb) e4m3 den8 = den/8     rcb2 = 8/den  -> an8 = 128*attn
  wo8 = 32*Wo.T               out_psum = 4096*out, residual add scales 1/4096

LayerNorm: per-token mean/var via ones-matmuls (cross-partition sum);
rstd/1024 from ACT Rsqrt (scale=2^20), invr via DVE (var+eps)*rstd; the
mean correction enters Q as a rank-2 bf16 matmul into the same PSUM group.

Softmax denominator: attn@V is augmented with a 1/8 ones column; the
denominator row is DMA-gathered into a [128,32] tile so one DVE
reciprocal covers all 8 heads, then DMA-broadcast back per head-pair.
attn@V PSUM is evicted by DMA (off-engine) and normalized straight into
the fp8 out-projection operand.
"""
import numpy as np
import ml_dtypes

from concourse import bacc, mybir, tile
from concourse.bass_utils import run_bass_kernel_spmd

BF = ml_dtypes.bfloat16
F8 = ml_dtypes.float8_e4m3

B, C, H, W = 16, 512, 32, 32
NH, HD = 8, 64
CTX_DIM, L = 768, 512
EPS = 1e-5
N = H * W                 # 1024 tokens
NCORES = 8
BPC = B // NCORES         # batches per core
P = 128
CC = C // P               # 4 c-chunks
DC = CTX_DIM // P         # 6 d-chunks
LC = L // P               # 4 l-chunks
MC = N // 512             # 2 token chunks of 512
MASK_NEG = -30000.0

F32 = mybir.dt.float32
BF16 = mybir.dt.bfloat16
FP8 = mybir.dt.float8e4
DR = mybir.MatmulPerfMode.DoubleRow

_NC_CACHE = None


def _build():
    nc = bacc.Bacc(None, target_bir_lowering=False, debug=False)

    x8d = nc.dram_tensor("x8", [BPC, C, N], FP8, kind="ExternalInput")
    xbfd = nc.dram_tensor("xbf", [BPC, C, N], BF16, kind="ExternalInput")
    ctx8d = nc.dram_tensor("ctx8", [BPC, CTX_DIM, L], FP8, kind="ExternalInput")
    # fp8 aug rows: mask row for k8 (i=0: -224 masked / 0, i=1: zeros) and
    # the constant 16 / 0 rows for q8
    mrowd = nc.dram_tensor("mrow8", [BPC, 1, NH, L], FP8, kind="ExternalInput")
    qaugd = nc.dram_tensor("qaug8", [1, NH, MC, 512], FP8, kind="ExternalInput")
    wq8d = nc.dram_tensor("wq8", [C, C], FP8, kind="ExternalInput")
    wck8d = nc.dram_tensor("wck8", [CTX_DIM, C], FP8, kind="ExternalInput")
    wcv8d = nc.dram_tensor("wcv8", [CTX_DIM, C], FP8, kind="ExternalInput")
    wo8d = nc.dram_tensor("wo8", [C, C], FP8, kind="ExternalInput")
    qr2d = nc.dram_tensor("q_r2", [2, C], BF16, kind="ExternalInput")
    yd = nc.dram_tensor("y", [BPC, C, N], F32, kind="ExternalOutput")

    with tile.TileContext(nc) as tc:
        with (
            tc.tile_pool(name="wpool", bufs=1) as wpool,
            tc.tile_pool(name="xpool", bufs=2) as xpool,
            tc.tile_pool(name="actpool", bufs=2) as actpool,
            tc.tile_pool(name="ppool", bufs=6) as ppool,
            tc.tile_pool(name="spool", bufs=2) as spool,
            tc.tile_pool(name="psum", bufs=2, space="PSUM") as psum,
            tc.tile_pool(name="psc", bufs=2, space="PSUM") as psc,
            tc.tile_pool(name="paug", bufs=2, space="PSUM") as paug,
            tc.tile_pool(name="dpool", bufs=4, space="DRAM") as dpool,
        ):
            # ---- persistent weights ----
            wq_sb = wpool.tile([P, CC, C], FP8)
            nc.scalar.dma_start(wq_sb[:], wq8d.ap().rearrange("(cc p) e -> p cc e", p=P))
            wck_sb = wpool.tile([P, DC, C], FP8)
            nc.scalar.dma_start(wck_sb[:], wck8d.ap().rearrange("(dc p) e -> p dc e", p=P))
            wcv_sb = wpool.tile([P, DC, C], FP8)
            nc.scalar.dma_start(wcv_sb[:], wcv8d.ap().rearrange("(dc p) e -> p dc e", p=P))
            wo_sb = wpool.tile([P, CC, C], FP8)
            nc.scalar.dma_start(wo_sb[:], wo8d.ap().rearrange("(ec p) c -> p ec c", p=P))
            qr2_sb = wpool.tile([2, C], BF16)
            nc.scalar.dma_start(qr2_sb[:], qr2d.ap())

            ones1_sb = wpool.tile([P, 1], BF16)   # stats lhsT (column sums)
            nc.vector.memset(ones1_sb[:], 1.0)
            onesr_sb = wpool.tile([1, P], BF16)    # bcast-matmul lhsT (rank-1)
            nc.vector.memset(onesr_sb[:], 1.0)
            eps2_sb = wpool.tile([1, 1], F32)      # eps * 2^8 (scaled Sqrt bias)
            nc.vector.memset(eps2_sb[:], EPS * 256.0)

            # Per-batch emission closures; emitted in a software-pipelined
            # order so PE filler (projection chains) sits between the
            # ACT-bound score-exp groups and their attn@v consumers.
            def make_batch(b):
                st = {}

                def loads():
                    # spread bulk loads across queues so ctx (feeds v/k), xbf
                    # (feeds stats) and x8 (feeds q) all stream in parallel;
                    # b0's ctx rides sync, later batches keep off sync so the
                    # previous batch's latency-sensitive normalize DMAs win.
                    bulk = nc.sync.dma_start if b == 0 else nc.gpsimd.dma_start
                    st["x8"] = xpool.tile([P, CC, N], FP8, name=f"x8{b}", tag="x8")
                    st["xbf"] = xpool.tile([P, CC, N], BF16, name=f"xbf{b}", tag="xbf")
                    st["ctx8"] = xpool.tile([P, DC, L], FP8, name=f"ctx8{b}", tag="ctx8")
                    for dc in range(DC):
                        bulk(st["ctx8"][:, dc, :],
                             ctx8d.ap()[b][dc * P:(dc + 1) * P, :])
                    for cc in range(CC):
                        nc.gpsimd.dma_start(st["xbf"][:, cc, :],
                                            xbfd.ap()[b][cc * P:(cc + 1) * P, :])
                    for cc in range(CC):
                        nc.scalar.dma_start(
                            st["x8"][:, cc, :],
                            x8d.ap()[b][cc * P:(cc + 1) * P, :])
                    # scores operands: [65, h, ...] fp8, aug row 64 carries the
                    # mask on the k side and a constant 16 on the q side.
                    # Head dims are host-interleaved so one [128,512] DMA
                    # fills both heads of an ec chunk (row r -> head r%2).
                    st["k8"] = actpool.tile([65, NH, L], FP8,
                                            name=f"k8{b}", tag="k8")
                    st["q8"] = actpool.tile([65, NH, MC, 512], FP8,
                                            name=f"q8{b}", tag="q8")
                    nc.sync.dma_start(st["k8"][64:65, :, :], mrowd.ap()[b])
                    nc.sync.dma_start(st["q8"][64:65, :, :, :], qaugd.ap()[0])
                    st["xsq"] = xpool.tile([P, CC, N], BF16, name=f"xsq{b}",
                                           tag="xsq", bufs=1)
                    # b0's xsq feeds the first stats directly: keep it on the
                    # fast DVE; later batches use the idle gpsimd
                    xsq_eng = nc.vector if b == 0 else nc.gpsimd
                    for cc in range(CC):
                        xsq_eng.tensor_tensor(
                            st["xsq"][:, cc, :], st["xbf"][:, cc, :],
                            st["xbf"][:, cc, :], op=mybir.AluOpType.mult)

                    # v8: [d, lc-pair u, k-tile i, head, 96]; col 64 = 1/8 ones
                    # (denominator), cols 65:96 zero pad (DoubleRow stationary
                    # width must be a multiple of 32)
                    st["v8"] = actpool.tile([P, LC // 2, 2, NH, 96], FP8,
                                            name=f"v8{b}", tag="v8")
                    nc.vector.memset(st["v8"][:, :, :, :, HD + 1:], 0.0)
                    nc.vector.memset(st["v8"][:, :, :, :, HD:HD + 1], 0.125)
                    st["an8"] = actpool.tile([P, CC, MC, 512], FP8,
                                             name=f"an8{b}", tag="an8")
                    st["r2"] = {}
                    st["rbc"] = {}
                    st["den"] = {}
                    st["asb"] = {}
                    st["rcb"] = {}

                def stats(mc):
                    ms = slice(mc * 512, (mc + 1) * 512)
                    st1 = psum.tile([1, 512], F32, name=f"st1{b}{mc}", tag="ps")
                    for cc in range(CC):
                        nc.tensor.matmul(st1[:], ones1_sb[:], st["xbf"][:, cc, ms],
                                         start=(cc == 0), stop=(cc == CC - 1))
                    st2 = psum.tile([1, 512], F32, name=f"st2{b}{mc}", tag="ps")
                    for cc in range(CC):
                        nc.tensor.matmul(st2[:], ones1_sb[:], st["xsq"][:, cc, ms],
                                         start=(cc == 0), stop=(cc == CC - 1))
                    negmu = spool.tile([1, 512], BF16, name=f"negmu{b}{mc}", tag="negmu")
                    nc.vector.tensor_scalar_mul(negmu[:], st1[:], -1.0 / C)
                    musq = spool.tile([1, 512], F32, name=f"musq{b}{mc}", tag="musq")
                    nc.vector.tensor_tensor(musq[:], negmu[:], negmu[:],
                                            op=mybir.AluOpType.mult)
                    var = spool.tile([1, 512], F32, name=f"var{b}{mc}", tag="var")
                    nc.vector.scalar_tensor_tensor(
                        var[:], st2[:], 1.0 / C, musq[:],
                        op0=mybir.AluOpType.mult, op1=mybir.AluOpType.subtract)
                    # invr_k = sqrt((var+eps)*2^8) = 16*invr  (fp32 for recip)
                    invr_k = spool.tile([1, 512], F32, name=f"invk{b}{mc}", tag="invk")
                    nc.scalar.activation(invr_k[:], var[:],
                                         mybir.ActivationFunctionType.Sqrt,
                                         bias=eps2_sb[:], scale=256.0)
                    rstd_f = spool.tile([1, 512], F32, name=f"rstf{b}{mc}", tag="rstf")
                    nc.vector.reciprocal_approx_fast(rstd_f[:], invr_k[:])
                    rstd = spool.tile([1, 512], BF16, name=f"rstd{b}{mc}", tag="rstd")
                    nc.vector.tensor_copy(rstd[:], rstd_f[:])
                    # invr/16 = (var+eps) * (rstd/16)
                    invr = spool.tile([1, 512], BF16, name=f"invr{b}{mc}", tag="invr")
                    nc.vector.scalar_tensor_tensor(
                        invr[:], var[:], EPS, rstd[:],
                        op0=mybir.AluOpType.add, op1=mybir.AluOpType.mult)
                    r2 = spool.tile([2, 512], BF16, name=f"r2_{b}{mc}", tag="r2")
                    nc.sync.dma_start(r2[0:1, :], negmu[:])
                    nc.sync.dma_start(r2[1:2, :], invr[:])
                    rbp = paug.tile([P, 512], F32, name=f"rbp{b}{mc}", tag="aug")
                    nc.tensor.matmul(rbp[:], onesr_sb[:], rstd[:],
                                     start=True, stop=True)
                    rbc = spool.tile([P, 512], BF16, name=f"rbc{b}{mc}", tag="rbc")
                    nc.vector.tensor_copy(rbc[:], rbp[:])
                    st["r2"][mc] = r2
                    st["rbc"][mc] = rbc

                def k_chain(ec):
                    es = slice(ec * P, (ec + 1) * P)
                    kp = psum.tile([P, 512], F32, name=f"kp{b}{ec}", tag="ps")
                    for u in range(DC // 2):
                        nc.tensor.matmul(kp[:], wck_sb[:, 2 * u:2 * u + 2, es],
                                         st["ctx8"][:, 2 * u:2 * u + 2, :],
                                         start=(u == 0), stop=(u == DC // 2 - 1),
                                         perf_mode=DR)
                    ktmp = ppool.tile([P, 512], FP8, name=f"ktmp{b}{ec}",
                                      tag="ktmp", bufs=3)
                    with nc.allow_low_precision(reason="fp8 attn operand; error damped by residual"):
                        nc.vector.tensor_copy(ktmp[:], kp[:])
                    # partition reshuffle rides the DMA engines
                    # (k8[p, 2ec + r%2] = ktmp[2p + r%2] via host interleave)
                    nc.gpsimd.dma_start(
                        st["k8"][0:HD, 2 * ec:2 * ec + 2, :], ktmp[:])

                def v_chain(lc):
                    ls = slice(lc * P, (lc + 1) * P)
                    vp = psum.tile([P, 512], F32, name=f"vp{b}{lc}", tag="ps")
                    for u in range(DC // 2):
                        nc.tensor.matmul(vp[:], st["ctx8"][:, 2 * u:2 * u + 2, ls],
                                         wcv_sb[:, 2 * u:2 * u + 2, :],
                                         start=(u == 0), stop=(u == DC // 2 - 1),
                                         perf_mode=DR)
                    with nc.allow_low_precision(reason="fp8 attn values; error damped by residual"):
                        nc.vector.tensor_copy(
                            st["v8"][:, lc // 2, lc % 2, :, 0:HD],
                            vp[:].rearrange("p (h d) -> p h d", d=HD))

                def q_chain(ec, mc):
                    es = slice(ec * P, (ec + 1) * P)
                    ms = slice(mc * 512, (mc + 1) * 512)
                    qp = psum.tile([P, 512], F32, name=f"qp{b}{ec}{mc}", tag="ps")
                    for u in range(CC // 2):
                        nc.tensor.matmul(qp[:], wq_sb[:, 2 * u:2 * u + 2, es],
                                         st["x8"][:, 2 * u:2 * u + 2, ms],
                                         start=(u == 0), stop=False,
                                         perf_mode=DR)
                    nc.tensor.matmul(qp[:], qr2_sb[:, es], st["r2"][mc][:],
                                     start=False, stop=True)
                    qtmp = ppool.tile([P, 512], FP8, name=f"qtmp{b}{ec}{mc}",
                                      tag="qtmp", bufs=3)
                    with nc.allow_low_precision(reason="fp8 attn operand; error damped by residual"):
                        nc.vector.tensor_tensor(qtmp[:], qp[:],
                                                st["rbc"][mc][:],
                                                op=mybir.AluOpType.mult)
                    nc.gpsimd.dma_start(
                        st["q8"][0:HD, 2 * ec:2 * ec + 2, mc, :], qtmp[:])

                def sc_exp_group(mc, j):
                    if mc not in st["den"]:
                        st["den"][mc] = [
                            spool.tile([4, 512], F32, name=f"den{b}{mc}{hf}",
                                       tag=f"den{hf}", bufs=2)
                            for hf in range(2)]
                        st["asb"][mc] = {}
                    # probs for this head-pair: per hh a [P, 2(u: lc pair),
                    # 2(i), 512] fp8 tile feeding the DoubleRow attn@v.
                    # scores are DoubleRow too ([33,2,*] k/q with the mask and
                    # a 16-const in aug row 32); exp applies the 1/64 descale.
                    ps_h = []
                    for hh in range(2):
                        h = 2 * j + hh
                        p8 = ppool.tile([P, 2, 2, 512], FP8,
                                        name=f"p8{b}{mc}{j}{hh}", tag="p8")
                        ts = [psc.tile([P, 2, 512], F32,
                                       name=f"sc{b}{mc}{j}{hh}{u}", tag="sc")
                              for u in range(2)]
                        for i in range(2):
                            for u in range(2):
                                lc = 2 * u + i
                                nc.tensor.matmul(
                                    ts[u][:, i, :],
                                    st["k8"][:, h, lc * P:(lc + 1) * P],
                                    st["q8"][:, h, mc, :],
                                    start=True, stop=True)
                        for u in range(2):
                            nc.scalar.activation(
                                p8[:, u, :, :], ts[u][:],
                                mybir.ActivationFunctionType.Exp,
                                scale=1.0 / 64.0)
                        ps_h.append(p8)
                    return ps_h

                def attnv_group(mc, j, ps_h):
                    for hh in range(2):
                        h = 2 * j + hh
                        aug = paug.tile([96, 512], F32,
                                        name=f"aug{b}{mc}{j}{hh}", tag="aug")
                        for u in range(2):
                            nc.tensor.matmul(aug[:], st["v8"][:, u, :, h, :],
                                             ps_h[hh][:, u, :, :],
                                             start=(u == 0), stop=(u == 1),
                                             perf_mode=DR)
                        asb = ppool.tile([HD + 1, 512], F32,
                                         name=f"asb{b}{mc}{j}{hh}", tag="asb",
                                         bufs=12)
                        nc.vector.tensor_copy(asb[:], aug[0:HD + 1, :])
                        nc.sync.dma_start(
                            st["den"][mc][h // 4][h % 4:h % 4 + 1, :],
                            asb[HD:HD + 1, :])
                        st["asb"][mc][h] = asb

                def norm(mc, half, tail=False):
                    # one 4-head half of the softmax normalize; halves are
                    # scheduled separately so the first overlaps its own pass.
                    # tail=True routes DMAs to the scalar queue (idle once the
                    # exps are done) so the epilogue doesn't drain behind sync.
                    dq = nc.scalar.dma_start if tail else nc.sync.dma_start
                    rcpf = spool.tile([4, 512], F32,
                                      name=f"rcpf{b}{mc}{half}", tag="rcpf")
                    nc.vector.reciprocal_approx_fast(rcpf[:],
                                                     st["den"][mc][half][:])
                    rcp = spool.tile([4, 512], BF16,
                                     name=f"rcp{b}{mc}{half}", tag="rcp")
                    nc.vector.tensor_copy(rcp[:], rcpf[:])
                    rcp_d = dpool.tile([4, 512], BF16,
                                       name=f"rcpd{b}{mc}{half}", tag="rcpd")
                    dq(rcp_d[:], rcp[:])
                    for h2 in range(4):
                        h = half * 4 + h2
                        j, hh = h // 2, h % 2
                        rcb = spool.tile([HD, 512], BF16,
                                         name=f"rcb{b}{mc}{h}", tag="rcb",
                                         bufs=6)
                        dq(rcb[:], rcp_d[h2:h2 + 1, :].to_broadcast((HD, 512)))
                        # split normalize between DVE and the idle gpsimd
                        eng = nc.vector if hh == 0 else nc.gpsimd
                        with nc.allow_low_precision(reason="fp8 attn operand; error damped by residual"):
                            eng.tensor_tensor(
                                st["an8"][hh * HD:(hh + 1) * HD, j, mc, :],
                                st["asb"][mc][h][0:HD, :], rcb[:],
                                op=mybir.AluOpType.mult)

                def outproj(cc, mc, tail=False):
                    ms = slice(mc * 512, (mc + 1) * 512)
                    cs = slice(cc * P, (cc + 1) * P)
                    op = psum.tile([P, 512], F32, name=f"op{b}{cc}{mc}", tag="ps")
                    for u in range(CC // 2):
                        nc.tensor.matmul(op[:], wo_sb[:, 2 * u:2 * u + 2, cs],
                                         st["an8"][:, 2 * u:2 * u + 2, mc, :],
                                         start=(u == 0), stop=(u == CC // 2 - 1),
                                         perf_mode=DR)
                    y_sb = xpool.tile([P, 512], F32, name=f"y{b}{cc}{mc}",
                                      tag="y", bufs=3)
                    nc.vector.scalar_tensor_tensor(
                        y_sb[:], op[:], 1.0 / 4096.0, st["xbf"][:, cc, ms],
                        op0=mybir.AluOpType.mult, op1=mybir.AluOpType.add)
                    dq = nc.scalar.dma_start if tail else nc.sync.dma_start
                    dq(yd.ap()[b][cc * P:(cc + 1) * P, mc * 512:(mc + 1) * 512],
                       y_sb[:])

                return dict(loads=loads, stats=stats, k_chain=k_chain,
                            v_chain=v_chain, q_chain=q_chain,
                            sc_exp_group=sc_exp_group, attnv_group=attnv_group,
                            norm=norm, outproj=outproj)

            # ---- software-pipelined emission (cross-batch modulo schedule) ----
            # PE fillers sit between ACT-bound score/exp groups and their
            # attn@v consumers; fillers are chosen to be independent of the
            # preceding normalize latency.
            E = [make_batch(b) for b in range(BPC)]

            def attn_pass(eb, mc, fillers, post_first=None):
                # attn@v trails the score/exp groups by one j so its P tiles
                # (ACT exps) are complete; fillers keep PE fed in between.
                # The first normalize half (heads 0-3) is emitted inside the
                # same pass once attnv(j=1) is out, so only heads 4-7's
                # denominator chain trails the pass.
                prev = None
                for j in range(NH // 2):
                    ps_h = eb["sc_exp_group"](mc, j)
                    fillers[j]()
                    if prev is not None:
                        eb["attnv_group"](mc, j - 1, prev)
                        if j == 1 and post_first is not None:
                            post_first()
                        if j == 2:
                            eb["norm"](mc, 0)
                    prev = ps_h
                eb["attnv_group"](mc, NH // 2 - 1, prev)

            def nop():
                pass

            E[0]["loads"]()
            E[0]["stats"](0)
            E[0]["stats"](1)
            for lc in range(LC):
                E[0]["v_chain"](lc)
            E[0]["k_chain"](0)
            E[0]["q_chain"](0, 0)
            E[0]["q_chain"](0, 1)

            def kq(eb, j):
                def f():
                    eb["k_chain"](j)
                    eb["q_chain"](j, 0)
                    eb["q_chain"](j, 1)
                return f

            attn_pass(E[0], 0, [kq(E[0], 1), kq(E[0], 2),
                                lambda: (E[1]["loads"](), kq(E[0], 3)()),
                                nop])
            attn_pass(E[0], 1,
                      [lambda: (E[1]["stats"](0), E[1]["stats"](1)),
                       lambda: (E[1]["v_chain"](0), E[1]["v_chain"](1)),
                       lambda: (E[1]["v_chain"](2), E[1]["v_chain"](3)),
                       kq(E[1], 0)],
                      post_first=lambda: E[0]["norm"](0, 1))
            attn_pass(E[1], 0,
                      [lambda: (E[0]["outproj"](0, 0), E[1]["k_chain"](1),
                                E[1]["q_chain"](1, 0), E[1]["q_chain"](1, 1)),
                       lambda: (E[0]["outproj"](1, 0), E[1]["k_chain"](2),
                                E[1]["q_chain"](2, 0), E[1]["q_chain"](2, 1)),
                       lambda: (E[0]["outproj"](2, 0), E[1]["k_chain"](3),
                                E[1]["q_chain"](3, 0), E[1]["q_chain"](3, 1)),
                       lambda: E[0]["outproj"](3, 0)],
                      post_first=lambda: E[0]["norm"](1, 1))
            attn_pass(E[1], 1,
                      [lambda: E[0]["outproj"](0, 1),
                       lambda: E[0]["outproj"](1, 1),
                       lambda: (E[0]["outproj"](2, 1), E[1]["outproj"](0, 0)),
                       lambda: (E[0]["outproj"](3, 1), E[1]["outproj"](1, 0))],
                      post_first=lambda: E[1]["norm"](0, 1))
            E[1]["outproj"](2, 0)
            E[1]["outproj"](3, 0)
            E[1]["norm"](1, 1, tail=True)
            for cc in range(CC):
                E[1]["outproj"](cc, 1, tail=True)
    nc.compile()
    return nc


def _get_nc():
    global _NC_CACHE
    if _NC_CACHE is None:
        _NC_CACHE = _build()
    return _NC_CACHE


def kernel(x, context, context_mask, ln_w, ln_b, Wq, Wk, Wv, Wo, Wctx):
    x = np.asarray(x, np.float32)
    context = np.asarray(context, np.float32)
    context_mask = np.asarray(context_mask)
    ln_w = np.asarray(ln_w, np.float32)
    ln_b = np.asarray(ln_b, np.float32)
    Wq = np.asarray(Wq, np.float32)
    Wk = np.asarray(Wk, np.float32)
    Wv = np.asarray(Wv, np.float32)
    Wo = np.asarray(Wo, np.float32)
    Wctx = np.asarray(Wctx, np.float32)

    scale = HD ** -0.5
    # interleave the two heads of each 128-wide e-chunk (row r -> head r%2,
    # dim r//2) so one [128,512] DMA reshuffles q/k into per-head layout
    perm = np.arange(C).reshape(CC, 2, HD).transpose(0, 2, 1).reshape(C)
    wq_f = Wq * (ln_w[None, :] * scale)          # [E, C] ln scale + attn scale
    wq8 = np.ascontiguousarray(wq_f.T[:, perm] * 64.0).astype(F8)
    q_r2 = np.stack([64.0 * wq_f.sum(1)[perm],
                     1024.0 * ((Wq * scale) @ ln_b)[perm]]).astype(BF)  # [2, E]
    wck8 = np.ascontiguousarray((Wk @ Wctx).T[:, perm] * 16.0).astype(F8)
    wcv8 = np.ascontiguousarray((Wv @ Wctx).T * 16.0).astype(F8)    # [768, 512]
    wo8 = np.ascontiguousarray(Wo.T * 32.0).astype(F8)

    xr = x.reshape(NCORES, BPC, C, N)
    x8 = xr.astype(F8)
    xbf = xr.astype(BF)
    ctx8 = np.ascontiguousarray(
        context.transpose(0, 2, 1)).astype(F8).reshape(NCORES, BPC, CTX_DIM, L)
    # k8 aug row 64: -224 on masked keys; paired with the constant-16 q aug
    # row and the 1/64 exp descale it contributes -56 -> exp() == 0
    mrow8 = np.broadcast_to(
        ((~context_mask).astype(np.float32) * -224.0)[:, None, None, :],
        (B, 1, NH, L)).astype(F8).reshape(NCORES, BPC, 1, NH, L)
    qaug8 = np.full((1, NH, MC, 512), 16.0, np.float32).astype(F8)

    in_maps = [
        {"x8": np.ascontiguousarray(x8[c]), "xbf": np.ascontiguousarray(xbf[c]),
         "ctx8": np.ascontiguousarray(ctx8[c]),
         "mrow8": np.ascontiguousarray(mrow8[c]), "qaug8": qaug8,
         "wq8": wq8, "wck8": wck8,
         "wcv8": wcv8, "wo8": wo8, "q_r2": q_r2}
        for c in range(NCORES)
    ]
    res = run_bass_kernel_spmd(_get_nc(), in_maps, core_ids=list(range(NCORES)))
    y = np.stack([r["y"] for r in res.results])          # [8, 2, C, N]
    return y.reshape(B, C, H, W)
